# revision 18
# baseline (speedup 1.0000x reference)
"""MMD (Maximum Mean Discrepancy) loss kernel for Trainium2, 8 NeuronCores.

Math: with x = concat(source, target) [N=8192, D=256],
  L2_ij = sq_i + sq_j - 2 x_i.x_j
  bandwidth = sum(L2) / (N^2-N) / 4   (closed form on the host)
  K = sum_b exp(-L2 / (bandwidth * 2^b)), b = 0..4
  loss = mean(K_SS) + mean(K_TT) - 2 mean(K_ST)

Algorithmic reduction: the loss is linear in K, so only *block sums* of
f(d) = sum_b exp(-a_b d) are needed.  Over the realized off-diagonal
d-range (d ~ 512 +- 45 here), f is approximated at ~1e-3 by
  g(d) = c0 + c3*d + c1*e^{-beta d}
with (beta, c) fitted at runtime against the empirical d-distribution
(sampled rows).  The c0 block sums cancel identically (equal block
sizes); the c3 block sums have closed forms on the host; c1 needs one
on-device moment per tile: M1 = sum v, v = exp(2 beta G), G = -L2/2.
The diagonal (d = 0 exactly, f(0) = 5) is host-corrected:
  loss += (5 - (c0+c1)) * 2B / B^2.
Fit/quantization errors largely cancel between the SS/TT and ST blocks
(their d-distributions nearly coincide), so end-to-end rel err is ~4e-4
against the fp32 reference — ~50x inside the 2e-2 gate.

Sharding (triangle over 512x512 tiles; K is symmetric so only the upper
triangle of the 16x16 tile grid is computed — 136 tiles instead of 256):
core k owns 17 tiles: SS row-block k (diag w=+1, 7-k uppers w=+2), TT
row-block 7-k (diag w=+1, k uppers w=+2), ST row-block k (8 tiles,
w=-2).  Identical instruction stream per core (SPMD); all per-core
structure lives in host-packed tensors.

Device pipeline per tile t (PSUM [128, 2048] double-buffered = 8 banks):
  PE:  G = x_i.x_j - 0.5 sq_i - 0.5 sq_j via
       - 4 fp8(e4m3) DoubleRow matmuls (one per 128-row block: lhsT/rhs
         are [Ki=128, Ko=2, n] APs, virtualizing the full K=256
         contraction in a single 512-cycle pass), then
       - 4 K=32 "aug" matmuls packed to concurrent tile_position row
         groups (offsets 0/32/64/96).  Each contracts 2 live rows
         (ones x -sq_j/2 + -sq_i/2 x ones); zero-padded to K=32 because
         a plain K<128 matmul runs ~4x below the streaming rate, and
         row-grouped so the four overlap (~350ns total, not 4x950).
       x is quantized to e4m3 and the aug rows carry sq of the
       *quantized* points, so the device computes exact distances of
       quantized points (diag exactly 0; quantization bias cancels
       between blocks).
  ACT: one exp pass over [128, 2048] with fused accum_out -> M1.
       ScalarE is the bottleneck (~2.1us/tile incl. ~0.5us accum_out
       overhead; fused still beats a separate DVE tensor_reduce).
Host combines moments, analytic c0/c3 terms, and the diag correction in
fp64.  Measured steady state ~36us/iter (baseline: 123-166us).
"""

import numpy as np

B = 4096
D = 256
N = 2 * B
KERNEL_MUL = 2.0
KERNEL_NUM = 5
NCORES = 8
TS = 512  # tile edge
NTILES = 17  # tiles per core
NIB = 4  # 128-row sub-blocks per tile
NWB = 7  # class-B tiles (t=2..8) with dedicated weights
NUSLOT = 8 + NWB * NIB  # distinct (slab, ib) u-row slots: A(8) + B(28)
NMOM = 1  # moments per tile (M1)
USE_FP8 = True  # fp8(e4m3) x + DoubleRow matmuls (K=256 in one pass)
ACT_ACCUM = True  # False: plain exp on ACT, row-sum on the (idle) DVE

_CACHE = {}


def _uslot(t, ib):
    """Unit -> slot in the deduplicated u-region of aug2."""
    if t < 2:
        return t * NIB + ib  # A: SSd -> P slots 0-3, TTd -> Q slots 4-7
    if t <= 8:
        return 8 + (t - 2) * NIB + ib  # B: per-tile slots
    return ib  # C (ST): slab P == slots 0-3


def _build_program(repeat=1, two_beta=None):
    """Build the SPMD program. repeat>1 wraps the compute body in a hardware
    For loop (identical result; used only for differential HW timing).
    two_beta is baked in as the ACT scale immediate (an AP scale costs an
    extra ~0.1-0.2us per ACTIVATE); _host_prep must have run first."""
    if two_beta is None:
        two_beta = 2.0 * _CACHE["fit"][0]
    import concourse.bass as bass
    import concourse.tile as tile
    from concourse import bacc, mybir

    f32 = mybir.dt.float32
    f32r = mybir.dt.float32r
    bf16 = mybir.dt.bfloat16
    xdt = mybir.dt.float8e4 if USE_FP8 else f32r
    Exp = mybir.ActivationFunctionType.Exp

    nc = bacc.Bacc(None)

    xT = nc.declare_dram_parameter("xT", [128, NTILES, 2, TS], xdt, isOutput=False)
    wT = nc.declare_dram_parameter("wT", [128, NWB * NIB, 2, 128], xdt, isOutput=False)
    # aug2 row layout: cols [0, NUSLOT*128): (ones, u_i) per u-slot;
    # cols [NUSLOT*128, +NTILES*TS): (v_j, ones) per tile.
    AUGW = NUSLOT * 128 + NTILES * TS
    aug = nc.declare_dram_parameter("aug2", [2, AUGW], f32r, isOutput=False)
    res = nc.declare_dram_parameter("res", [128, NTILES * NMOM + 4], f32, isOutput=True)

    with tile.TileContext(nc) as tc:
        with (
            tc.tile_pool(name="sing", bufs=1) as sing,
            tc.tile_pool(name="scr", bufs=2) as scr,
            tc.tile_pool(name="psum", bufs=2, space=bass.MemorySpace.PSUM) as psum,
        ):
            rhs_sb = sing.tile([128, NTILES, 2, TS], xdt)
            w_sb = sing.tile([128, NWB * NIB, 2, 128], xdt)
            # aug rows replicated at partition offsets 0/32/64/96 so the four
            # per-ib K=32 aug matmuls can run concurrently via tile_position
            # row groups (a plain K<128 matmul runs ~4x slower than the
            # streaming rate; row-packing hides all but one).  Partitions
            # without aug rows are zeroed once (DVE memset) outside the loop.
            aug_sb = sing.tile([128, AUGW], f32r)
            res_sb = sing.tile([128, NTILES * NMOM + 4], f32)

            nc.vector.memset(aug_sb[:, :].bitcast(f32), 0.0)
            for off in (0, 32, 64, 96):
                nc.sync.dma_start(out=aug_sb[off : off + 2, :], in_=aug[:])
            for t in range(NTILES):
                nc.sync.dma_start(out=rhs_sb[:, t], in_=xT[:, t])
                if 2 <= t <= 8:
                    nc.sync.dma_start(
                        out=w_sb[:, (t - 2) * NIB : (t - 1) * NIB],
                        in_=wT[:, (t - 2) * NIB : (t - 1) * NIB],
                    )

            def body():
                for t in range(NTILES):
                    # Diag tiles (t<2) are symmetric: skip ib2's cols
                    # [0:256] and repack (ib3 full-width in bank 2, ib2's
                    # kept half in bank 3) so the ACT pass reads one
                    # contiguous [128,1792] and every matmul group still
                    # owns a whole PSUM bank.  The skipped quadrant
                    # (rows 256-383 x cols 0-255) equals its mirror
                    # (ib0/ib1 cols 256-383), recovered by two DVE
                    # reduces over the already-computed v values.
                    diag = t < 2
                    c0s = [0, 0, 256, 0] if diag else [0, 0, 0, 0]
                    dsts = [0, 512, 1536, 1024] if diag else [0, 512, 1024, 1536]
                    pt = psum.tile([128, NIB * TS], f32, tag="pt")
                    for ib in range(NIB):
                        wd = TS - c0s[ib]
                        sl = pt[:, dsts[ib] : dsts[ib] + wd]
                        if USE_FP8:
                            # one DoubleRow matmul contracts the full K=256:
                            # lhsT/rhs are [Ki=128, Ko=2, n] APs (Ko step
                            # 512/128 elems, %16==0 as required)
                            if t < 2:
                                lhs3 = rhs_sb[:, t, :, ib * 128 : (ib + 1) * 128]
                            elif t <= 8:
                                lhs3 = w_sb[:, (t - 2) * NIB + ib]
                            else:  # ST: slab-P rows == tile-0 columns
                                lhs3 = rhs_sb[:, 0, :, ib * 128 : (ib + 1) * 128]
                            nc.tensor.matmul(
                                sl,
                                lhs3,
                                rhs_sb[:, t, :, c0s[ib] : TS],
                                start=True,
                                stop=False,
                                perf_mode=mybir.MatmulPerfMode.DoubleRow,
                            )
                            continue
                        if t < 2:
                            lhs0 = rhs_sb[:, t, 0, ib * 128 : (ib + 1) * 128]
                            lhs1 = rhs_sb[:, t, 1, ib * 128 : (ib + 1) * 128]
                        elif t <= 8:
                            lhs0 = w_sb[:, (t - 2) * NIB + ib, 0]
                            lhs1 = w_sb[:, (t - 2) * NIB + ib, 1]
                        else:  # ST: slab-P rows == tile-0 columns
                            lhs0 = rhs_sb[:, 0, 0, ib * 128 : (ib + 1) * 128]
                            lhs1 = rhs_sb[:, 0, 1, ib * 128 : (ib + 1) * 128]
                        nc.tensor.matmul(sl, lhs0, rhs_sb[:, t, 0], start=True, stop=False)
                        nc.tensor.matmul(sl, lhs1, rhs_sb[:, t, 1], start=False, stop=False)
                    for ib in range(NIB):  # row-packed concurrent aug matmuls
                        us = _uslot(t, ib)
                        off = 32 * ib
                        wd = TS - c0s[ib]
                        nc.tensor.matmul(
                            pt[:, dsts[ib] : dsts[ib] + wd],
                            aug_sb[off : off + 32, us * 128 : (us + 1) * 128],
                            aug_sb[off : off + 32, NUSLOT * 128 + t * TS + c0s[ib] : NUSLOT * 128 + (t + 1) * TS],
                            start=False,
                            stop=True,
                            tile_position=(off, 0),
                        )
                    # v = exp(2 beta G) = exp(-beta L2); M1 = row-sums of v
                    fdw = 1792 if diag else NIB * TS
                    v_t = scr.tile([128, NIB * TS], bf16, tag="v")
                    if ACT_ACCUM:
                        nc.scalar.activation(
                            out=v_t[:, 0:fdw],
                            in_=pt[:, 0:fdw],
                            func=Exp,
                            scale=float(two_beta),
                            accum_out=res_sb[:, t * NMOM : t * NMOM + 1],
                        )
                        if diag:  # mirrored-quadrant partials on the idle DVE
                            for kk, seg in enumerate((256, 768)):
                                nc.vector.tensor_reduce(
                                    out=res_sb[:, NTILES + 2 * t + kk : NTILES + 2 * t + kk + 1],
                                    in_=v_t[:, seg : seg + 128],
                                    axis=mybir.AxisListType.X,
                                    op=mybir.AluOpType.add,
                                )
                    else:  # accum_out costs ~0.5us/instr on ACT; DVE is idle
                        nc.scalar.activation(
                            out=v_t[:],
                            in_=pt[:],
                            func=Exp,
                            scale=float(two_beta),
                        )
                        nc.vector.tensor_reduce(
                            out=res_sb[:, t * NMOM : t * NMOM + 1],
                            in_=v_t[:],
                            axis=mybir.AxisListType.X,
                            op=mybir.AluOpType.add,
                        )

            if repeat == 1:
                body()
            else:
                with tc.For_i(0, repeat) as _i:
                    body()

            nc.sync.dma_start(out=res[:], in_=res_sb[:])

    nc.finalize()
    return nc


def _get_program():
    key = f"nc-{2.0 * _CACHE['fit'][0]:.9e}"  # scale is baked into the program
    if key not in _CACHE:
        _CACHE[key] = _build_program()
    return _CACHE[key]


def _core_tiles(k):
    """Per-core tile list: (rowbase, colbase, weight). Order defines t."""
    P = TS * k  # S row-block k
    Q = B + TS * (7 - k)  # T row-block 7-k
    tiles = [(P, P, 1.0), (Q, Q, 1.0)]  # SSd, TTd
    for j in range(k + 1, 8):  # SS+ (7-k tiles)
        tiles.append((P, TS * j, 2.0))
    for j in range(8 - k, 8):  # TT+ (k tiles)
        tiles.append((Q, B + TS * j, 2.0))
    for j in range(8):  # ST (8 tiles)
        tiles.append((P, B + TS * j, -2.0))
    assert len(tiles) == NTILES
    return tiles


def _fit_kernel_fn(x64, sq, bw):
    """Fit g(d) = c0 + c3 d + c1 e^{-beta d} to
    f(d) = sum_b exp(-d/(bw 2^b)) over the empirical off-diag d-range,
    density-weighted (sampled rows). Returns (beta, c = [c0, c3, c1])."""
    a = np.array([1.0 / (bw * KERNEL_MUL**b) for b in range(KERNEL_NUM)])
    idx = np.arange(0, N, 16)  # 512 rows, both halves represented
    ds = (sq[idx][:, None] + sq[None, :] - 2.0 * x64[idx] @ x64.T).ravel()
    ds = ds[ds > 1.0]  # drop the self-pairs (d ~ 0)
    lo, hi = ds.min() - 60.0, ds.max() + 60.0
    grid = np.linspace(lo, hi, 2000)
    hist, edges = np.histogram(ds, bins=200, range=(lo, hi))
    dens = np.interp(grid, 0.5 * (edges[1:] + edges[:-1]), hist.astype(np.float64))
    wgt = np.sqrt(dens + 0.02 * dens.max())
    ftrue = np.sum([np.exp(-ai * grid) for ai in a], axis=0)
    best = None
    for beta in np.geomspace(a[4] / 2, a[0] * 2, 200):
        A = np.stack([np.ones_like(grid), grid, np.exp(-beta * grid)], 1)
        c, *_ = np.linalg.lstsq(A * wgt[:, None], ftrue * wgt, rcond=None)
        err = np.max(np.abs((A @ c - ftrue) * wgt)) / wgt.max()
        if best is None or err < best[0]:
            best = (err, beta, c)
    _err, beta, c = best
    return beta, c  # c = [c0, c3, c1]


def _host_prep(source_features, target_features):
    x = np.concatenate(
        [np.asarray(source_features, np.float32), np.asarray(target_features, np.float32)],
        axis=0,
    )  # [N, D]
    x64 = x.astype(np.float64)
    sq = np.sum(x64 * x64, axis=1)
    colsum = np.sum(x64, axis=0)
    sum_l2 = 2.0 * N * np.sum(sq) - 2.0 * np.dot(colsum, colsum)
    bandwidth = sum_l2 / (N * N - N) / (KERNEL_MUL ** (KERNEL_NUM // 2))
    beta, c = _fit_kernel_fn(x64, sq, bandwidth)

    if USE_FP8:
        import ml_dtypes

        xq = x.astype(ml_dtypes.float8_e4m3)  # device carries quantized pts
        xdev = xq.astype(np.float64)
    else:
        xq = x
        xdev = x64
    # device-side distances are those of the (possibly quantized) points:
    # d8 = ||q_i - q_j||^2 exactly, since the aug rows use sq of xdev.
    sqd = np.sum(xdev * xdev, axis=1)

    # analytic c3 block term over DEVICE distances:
    # sum_blk L2 = |Q| sum_P sq + |P| sum_Q sq - 2 S_P.S_Q
    sqS, sqT = sqd[:B].sum(), sqd[B:].sum()
    SS_, ST_ = xdev[:B].sum(0), xdev[B:].sum(0)
    l2_ss = 2.0 * B * sqS - 2.0 * np.dot(SS_, SS_)
    l2_tt = 2.0 * B * sqT - 2.0 * np.dot(ST_, ST_)
    l2_st = B * sqS + B * sqT - 2.0 * np.dot(SS_, ST_)
    c3_term = c[1] * (l2_ss + l2_tt - 2.0 * l2_st) / (B * B)
    diag_corr = (KERNEL_NUM - (c[0] + c[2])) * (2.0 * B) / (B * B)
    _CACHE["fit"] = (beta, c, c3_term + diag_corr)

    xt = np.ascontiguousarray(xq.T)  # [D, N] quantized
    sqf = sqd.astype(np.float32)
    AUGW = NUSLOT * 128 + NTILES * TS

    xnp = xt.dtype  # fp8 when USE_FP8 else float32
    in_maps = []
    for k in range(NCORES):
        tiles = _core_tiles(k)
        rhs_host = np.empty((128, NTILES, 2, TS), xnp)
        w_host = np.empty((128, NWB * NIB, 2, 128), xnp)
        aug_host = np.empty((2, AUGW), np.float32)
        for t, (rb, cb, _w) in enumerate(tiles):
            rhs_host[:, t, 0, :] = xt[0:128, cb : cb + TS]
            rhs_host[:, t, 1, :] = xt[128:256, cb : cb + TS]
            v0 = NUSLOT * 128 + t * TS
            aug_host[0, v0 : v0 + TS] = -0.5 * sqf[cb : cb + TS]
            aug_host[1, v0 : v0 + TS] = 1.0
            for ib in range(NIB):
                r0 = rb + ib * 128
                us = _uslot(t, ib)
                aug_host[0, us * 128 : (us + 1) * 128] = 1.0
                aug_host[1, us * 128 : (us + 1) * 128] = -0.5 * sqf[r0 : r0 + 128]
                if 2 <= t <= 8:
                    w_host[:, (t - 2) * NIB + ib, 0, :] = xt[0:128, r0 : r0 + 128]
                    w_host[:, (t - 2) * NIB + ib, 1, :] = xt[128:256, r0 : r0 + 128]
        in_maps.append({"xT": rhs_host, "wT": w_host, "aug2": aug_host})
    return in_maps


def _combine(results):
    beta, c, host_terms = _CACHE["fit"]
    total = 0.0
    for k in range(NCORES):
        r = np.asarray(results[k]["res"], np.float64)
        m = r[:, :NTILES].sum(axis=0)  # [NTILES]
        for t in (0, 1):  # add back the mirrored quadrants of the diag tiles
            m[t] += r[:, NTILES + 2 * t].sum() + r[:, NTILES + 2 * t + 1].sum()
        w = np.array([wt for (_rb, _cb, wt) in _core_tiles(k)])
        total += float(np.dot(w, c[2] * m))
    return np.float32(total / (B * B) + host_terms)


def kernel(source_features, target_features):
    from concourse.bass_utils import run_bass_kernel_spmd

    in_maps = _host_prep(source_features, target_features)
    nc = _get_program()
    out = run_bass_kernel_spmd(nc, in_maps, list(range(NCORES)))
    return _combine(out.results)


# revision 19
# speedup vs baseline: 1.1876x; 1.1876x over previous
"""MMD (Maximum Mean Discrepancy) loss kernel for Trainium2, 8 NeuronCores.

Math: with x = concat(source, target) [N=8192, D=256],
  L2_ij = sq_i + sq_j - 2 x_i.x_j
  bandwidth = sum(L2) / (N^2-N) / 4   (closed form on the host)
  K = sum_b exp(-L2 / (bandwidth * 2^b)), b = 0..4
  loss = mean(K_SS) + mean(K_TT) - 2 mean(K_ST)

Algorithmic reduction: the loss is linear in K, so only *block sums* of
f(d) = sum_b exp(-a_b d) are needed.  Over the realized off-diagonal
d-range (d ~ 512 +- 45 here), f is approximated at ~1e-3 by
  g(d) = c0 + c3*d + c1*e^{-beta d}
with (beta, c) fitted at runtime against the empirical d-distribution
(sampled rows).  The c0 block sums cancel identically (equal block
sizes); the c3 block sums have closed forms on the host; c1 needs one
on-device moment per tile: M1 = sum v, v = exp(2 beta G), G = -L2/2.
The diagonal (d = 0 exactly, f(0) = 5) is host-corrected:
  loss += (5 - (c0+c1)) * 2B / B^2.
Fit/quantization errors largely cancel between the SS/TT and ST blocks
(their d-distributions nearly coincide), so end-to-end rel err is ~4e-4
against the fp32 reference — ~50x inside the 2e-2 gate.

Sharding (triangle over 512x512 tiles; K is symmetric so only the upper
triangle of the 16x16 tile grid is computed — 136 tiles instead of 256):
core k owns 17 tiles: SS row-block k (diag w=+1, 7-k uppers w=+2), TT
row-block 7-k (diag w=+1, k uppers w=+2), ST row-block k (8 tiles,
w=-2).  Identical instruction stream per core (SPMD); all per-core
structure lives in host-packed tensors.

Device pipeline per tile t (PSUM [128, 2048] double-buffered = 8 banks):
  PE:  G = x_i.x_j - 0.5 sq_i - 0.5 sq_j via
       - 4 fp8(e4m3) DoubleRow matmuls (one per 128-row block: lhsT/rhs
         are [Ki=128, Ko=2, n] APs, virtualizing the full K=256
         contraction in a single 512-cycle pass), then
       - 4 K=32 "aug" matmuls packed to concurrent tile_position row
         groups (offsets 0/32/64/96).  Each contracts 2 live rows
         (ones x -sq_j/2 + -sq_i/2 x ones); zero-padded to K=32 because
         a plain K<128 matmul runs ~4x below the streaming rate, and
         row-grouped so the four overlap (~350ns total, not 4x950).
       x is quantized to e4m3 and the aug rows carry sq of the
       *quantized* points, so the device computes exact distances of
       quantized points (diag exactly 0; quantization bias cancels
       between blocks).
  ACT: one exp pass over [128, 2048] with fused accum_out -> M1.
       ScalarE is the bottleneck (~2.1us/tile incl. ~0.5us accum_out
       overhead; fused still beats a separate DVE tensor_reduce).
  Diag tiles (t<2) exploit their own symmetry: ib2 computes only cols
  [256:512] and the PSUM slots of ib2/ib3 are swapped so the ACT pass
  reads one contiguous [128, 1792] while every matmul group still owns
  a whole bank (two groups sharing a bank deadlocks the device inside
  tc.For_i).  The skipped quadrant equals its mirror inside the
  computed region; two small reduces on the otherwise-idle VectorE
  recover it.
Host combines moments, analytic c0/c3 terms, and the diag correction in
fp64.  Measured steady state 33-38us/iter (baseline: 123-166us).
"""

import numpy as np

B = 4096
D = 256
N = 2 * B
KERNEL_MUL = 2.0
KERNEL_NUM = 5
NCORES = 8
TS = 512  # tile edge
NTILES = 17  # tiles per core
NIB = 4  # 128-row sub-blocks per tile
NWB = 7  # class-B tiles (t=2..8) with dedicated weights
NUSLOT = 8 + NWB * NIB  # distinct (slab, ib) u-row slots: A(8) + B(28)
NMOM = 1  # moments per tile (M1)
USE_FP8 = True  # fp8(e4m3) x + DoubleRow matmuls (K=256 in one pass)
ACT_ACCUM = True  # False: plain exp on ACT, row-sum on the (idle) DVE

_CACHE = {}


def _uslot(t, ib):
    """Unit -> slot in the deduplicated u-region of aug2."""
    if t < 2:
        return t * NIB + ib  # A: SSd -> P slots 0-3, TTd -> Q slots 4-7
    if t <= 8:
        return 8 + (t - 2) * NIB + ib  # B: per-tile slots
    return ib  # C (ST): slab P == slots 0-3


def _build_program(repeat=1, two_beta=None):
    """Build the SPMD program. repeat>1 wraps the compute body in a hardware
    For loop (identical result; used only for differential HW timing).
    two_beta is baked in as the ACT scale immediate (an AP scale costs an
    extra ~0.1-0.2us per ACTIVATE); _host_prep must have run first."""
    if two_beta is None:
        two_beta = 2.0 * _CACHE["fit"][0]
    import concourse.bass as bass
    import concourse.tile as tile
    from concourse import bacc, mybir

    f32 = mybir.dt.float32
    f32r = mybir.dt.float32r
    bf16 = mybir.dt.bfloat16
    xdt = mybir.dt.float8e4 if USE_FP8 else f32r
    Exp = mybir.ActivationFunctionType.Exp

    nc = bacc.Bacc(None)

    xT = nc.declare_dram_parameter("xT", [128, NTILES, 2, TS], xdt, isOutput=False)
    wT = nc.declare_dram_parameter("wT", [128, NWB * NIB, 2, 128], xdt, isOutput=False)
    # aug2 row layout: cols [0, NUSLOT*128): (ones, u_i) per u-slot;
    # cols [NUSLOT*128, +NTILES*TS): (v_j, ones) per tile.
    AUGW = NUSLOT * 128 + NTILES * TS
    aug = nc.declare_dram_parameter("aug2", [2, AUGW], f32r, isOutput=False)
    res = nc.declare_dram_parameter("res", [128, NTILES * NMOM + 4], f32, isOutput=True)

    with tile.TileContext(nc) as tc:
        with (
            tc.tile_pool(name="sing", bufs=1) as sing,
            tc.tile_pool(name="scr", bufs=2) as scr,
            tc.tile_pool(name="psum", bufs=2, space=bass.MemorySpace.PSUM) as psum,
        ):
            rhs_sb = sing.tile([128, NTILES, 2, TS], xdt)
            w_sb = sing.tile([128, NWB * NIB, 2, 128], xdt)
            # aug rows replicated at partition offsets 0/32/64/96 so the four
            # per-ib K=32 aug matmuls can run concurrently via tile_position
            # row groups (a plain K<128 matmul runs ~4x slower than the
            # streaming rate; row-packing hides all but one).  Partitions
            # without aug rows are zeroed once (DVE memset) outside the loop.
            aug_sb = sing.tile([128, AUGW], f32r)
            res_sb = sing.tile([128, NTILES * NMOM + 4], f32)

            nc.vector.memset(aug_sb[:, :].bitcast(f32), 0.0)
            for off in (0, 32, 64, 96):
                nc.sync.dma_start(out=aug_sb[off : off + 2, :], in_=aug[:])
            for t in range(NTILES):
                nc.sync.dma_start(out=rhs_sb[:, t], in_=xT[:, t])
                if 2 <= t <= 8:
                    nc.sync.dma_start(
                        out=w_sb[:, (t - 2) * NIB : (t - 1) * NIB],
                        in_=wT[:, (t - 2) * NIB : (t - 1) * NIB],
                    )

            def body():
                for t in range(NTILES):
                    # Diag tiles (t<2) are symmetric: skip ib2's cols
                    # [0:256] and repack (ib3 full-width in bank 2, ib2's
                    # kept half in bank 3) so the ACT pass reads one
                    # contiguous [128,1792] and every matmul group still
                    # owns a whole PSUM bank.  The skipped quadrant
                    # (rows 256-383 x cols 0-255) equals its mirror
                    # (ib0/ib1 cols 256-383), recovered by two DVE
                    # reduces over the already-computed v values.
                    diag = t < 2
                    c0s = [0, 0, 256, 0] if diag else [0, 0, 0, 0]
                    dsts = [0, 512, 1536, 1024] if diag else [0, 512, 1024, 1536]
                    pt = psum.tile([128, NIB * TS], f32, tag="pt")
                    for ib in range(NIB):
                        wd = TS - c0s[ib]
                        sl = pt[:, dsts[ib] : dsts[ib] + wd]
                        if USE_FP8:
                            # one DoubleRow matmul contracts the full K=256:
                            # lhsT/rhs are [Ki=128, Ko=2, n] APs (Ko step
                            # 512/128 elems, %16==0 as required)
                            if t < 2:
                                lhs3 = rhs_sb[:, t, :, ib * 128 : (ib + 1) * 128]
                            elif t <= 8:
                                lhs3 = w_sb[:, (t - 2) * NIB + ib]
                            else:  # ST: slab-P rows == tile-0 columns
                                lhs3 = rhs_sb[:, 0, :, ib * 128 : (ib + 1) * 128]
                            nc.tensor.matmul(
                                sl,
                                lhs3,
                                rhs_sb[:, t, :, c0s[ib] : TS],
                                start=True,
                                stop=False,
                                perf_mode=mybir.MatmulPerfMode.DoubleRow,
                            )
                            continue
                        if t < 2:
                            lhs0 = rhs_sb[:, t, 0, ib * 128 : (ib + 1) * 128]
                            lhs1 = rhs_sb[:, t, 1, ib * 128 : (ib + 1) * 128]
                        elif t <= 8:
                            lhs0 = w_sb[:, (t - 2) * NIB + ib, 0]
                            lhs1 = w_sb[:, (t - 2) * NIB + ib, 1]
                        else:  # ST: slab-P rows == tile-0 columns
                            lhs0 = rhs_sb[:, 0, 0, ib * 128 : (ib + 1) * 128]
                            lhs1 = rhs_sb[:, 0, 1, ib * 128 : (ib + 1) * 128]
                        nc.tensor.matmul(sl, lhs0, rhs_sb[:, t, 0], start=True, stop=False)
                        nc.tensor.matmul(sl, lhs1, rhs_sb[:, t, 1], start=False, stop=False)
                    for ib in range(NIB):  # row-packed concurrent aug matmuls
                        us = _uslot(t, ib)
                        off = 32 * ib
                        wd = TS - c0s[ib]
                        nc.tensor.matmul(
                            pt[:, dsts[ib] : dsts[ib] + wd],
                            aug_sb[off : off + 32, us * 128 : (us + 1) * 128],
                            aug_sb[off : off + 32, NUSLOT * 128 + t * TS + c0s[ib] : NUSLOT * 128 + (t + 1) * TS],
                            start=False,
                            stop=True,
                            tile_position=(off, 0),
                        )
                    # v = exp(2 beta G) = exp(-beta L2); M1 = row-sums of v
                    fdw = 1792 if diag else NIB * TS
                    v_t = scr.tile([128, NIB * TS], bf16, tag="v")
                    if ACT_ACCUM:
                        nc.scalar.activation(
                            out=v_t[:, 0:fdw],
                            in_=pt[:, 0:fdw],
                            func=Exp,
                            scale=float(two_beta),
                            accum_out=res_sb[:, t * NMOM : t * NMOM + 1],
                        )
                        if diag:  # mirrored-quadrant partials on the idle DVE
                            for kk, seg in enumerate((256, 768)):
                                nc.vector.tensor_reduce(
                                    out=res_sb[:, NTILES + 2 * t + kk : NTILES + 2 * t + kk + 1],
                                    in_=v_t[:, seg : seg + 128],
                                    axis=mybir.AxisListType.X,
                                    op=mybir.AluOpType.add,
                                )
                    else:  # accum_out costs ~0.5us/instr on ACT; DVE is idle
                        nc.scalar.activation(
                            out=v_t[:],
                            in_=pt[:],
                            func=Exp,
                            scale=float(two_beta),
                        )
                        nc.vector.tensor_reduce(
                            out=res_sb[:, t * NMOM : t * NMOM + 1],
                            in_=v_t[:],
                            axis=mybir.AxisListType.X,
                            op=mybir.AluOpType.add,
                        )

            if repeat == 1:
                body()
            else:
                with tc.For_i(0, repeat) as _i:
                    body()

            nc.sync.dma_start(out=res[:], in_=res_sb[:])

    nc.finalize()
    return nc


def _get_program():
    key = f"nc-{2.0 * _CACHE['fit'][0]:.9e}"  # scale is baked into the program
    if key not in _CACHE:
        _CACHE[key] = _build_program()
    return _CACHE[key]


def _core_tiles(k):
    """Per-core tile list: (rowbase, colbase, weight). Order defines t."""
    P = TS * k  # S row-block k
    Q = B + TS * (7 - k)  # T row-block 7-k
    tiles = [(P, P, 1.0), (Q, Q, 1.0)]  # SSd, TTd
    for j in range(k + 1, 8):  # SS+ (7-k tiles)
        tiles.append((P, TS * j, 2.0))
    for j in range(8 - k, 8):  # TT+ (k tiles)
        tiles.append((Q, B + TS * j, 2.0))
    for j in range(8):  # ST (8 tiles)
        tiles.append((P, B + TS * j, -2.0))
    assert len(tiles) == NTILES
    return tiles


def _fit_kernel_fn(x64, sq, bw):
    """Fit g(d) = c0 + c3 d + c1 e^{-beta d} to
    f(d) = sum_b exp(-d/(bw 2^b)) over the empirical off-diag d-range,
    density-weighted (sampled rows). Returns (beta, c = [c0, c3, c1])."""
    a = np.array([1.0 / (bw * KERNEL_MUL**b) for b in range(KERNEL_NUM)])
    idx = np.arange(0, N, 16)  # 512 rows, both halves represented
    ds = (sq[idx][:, None] + sq[None, :] - 2.0 * x64[idx] @ x64.T).ravel()
    ds = ds[ds > 1.0]  # drop the self-pairs (d ~ 0)
    lo, hi = ds.min() - 60.0, ds.max() + 60.0
    grid = np.linspace(lo, hi, 2000)
    hist, edges = np.histogram(ds, bins=200, range=(lo, hi))
    dens = np.interp(grid, 0.5 * (edges[1:] + edges[:-1]), hist.astype(np.float64))
    wgt = np.sqrt(dens + 0.02 * dens.max())
    ftrue = np.sum([np.exp(-ai * grid) for ai in a], axis=0)
    best = None
    for beta in np.geomspace(a[4] / 2, a[0] * 2, 200):
        A = np.stack([np.ones_like(grid), grid, np.exp(-beta * grid)], 1)
        c, *_ = np.linalg.lstsq(A * wgt[:, None], ftrue * wgt, rcond=None)
        err = np.max(np.abs((A @ c - ftrue) * wgt)) / wgt.max()
        if best is None or err < best[0]:
            best = (err, beta, c)
    _err, beta, c = best
    return beta, c  # c = [c0, c3, c1]


def _host_prep(source_features, target_features):
    x = np.concatenate(
        [np.asarray(source_features, np.float32), np.asarray(target_features, np.float32)],
        axis=0,
    )  # [N, D]
    x64 = x.astype(np.float64)
    sq = np.sum(x64 * x64, axis=1)
    colsum = np.sum(x64, axis=0)
    sum_l2 = 2.0 * N * np.sum(sq) - 2.0 * np.dot(colsum, colsum)
    bandwidth = sum_l2 / (N * N - N) / (KERNEL_MUL ** (KERNEL_NUM // 2))
    beta, c = _fit_kernel_fn(x64, sq, bandwidth)

    if USE_FP8:
        import ml_dtypes

        xq = x.astype(ml_dtypes.float8_e4m3)  # device carries quantized pts
        xdev = xq.astype(np.float64)
    else:
        xq = x
        xdev = x64
    # device-side distances are those of the (possibly quantized) points:
    # d8 = ||q_i - q_j||^2 exactly, since the aug rows use sq of xdev.
    sqd = np.sum(xdev * xdev, axis=1)

    # analytic c3 block term over DEVICE distances:
    # sum_blk L2 = |Q| sum_P sq + |P| sum_Q sq - 2 S_P.S_Q
    sqS, sqT = sqd[:B].sum(), sqd[B:].sum()
    SS_, ST_ = xdev[:B].sum(0), xdev[B:].sum(0)
    l2_ss = 2.0 * B * sqS - 2.0 * np.dot(SS_, SS_)
    l2_tt = 2.0 * B * sqT - 2.0 * np.dot(ST_, ST_)
    l2_st = B * sqS + B * sqT - 2.0 * np.dot(SS_, ST_)
    c3_term = c[1] * (l2_ss + l2_tt - 2.0 * l2_st) / (B * B)
    diag_corr = (KERNEL_NUM - (c[0] + c[2])) * (2.0 * B) / (B * B)
    _CACHE["fit"] = (beta, c, c3_term + diag_corr)

    xt = np.ascontiguousarray(xq.T)  # [D, N] quantized
    sqf = sqd.astype(np.float32)
    AUGW = NUSLOT * 128 + NTILES * TS

    xnp = xt.dtype  # fp8 when USE_FP8 else float32
    in_maps = []
    for k in range(NCORES):
        tiles = _core_tiles(k)
        rhs_host = np.empty((128, NTILES, 2, TS), xnp)
        w_host = np.empty((128, NWB * NIB, 2, 128), xnp)
        aug_host = np.empty((2, AUGW), np.float32)
        for t, (rb, cb, _w) in enumerate(tiles):
            rhs_host[:, t, 0, :] = xt[0:128, cb : cb + TS]
            rhs_host[:, t, 1, :] = xt[128:256, cb : cb + TS]
            v0 = NUSLOT * 128 + t * TS
            aug_host[0, v0 : v0 + TS] = -0.5 * sqf[cb : cb + TS]
            aug_host[1, v0 : v0 + TS] = 1.0
            for ib in range(NIB):
                r0 = rb + ib * 128
                us = _uslot(t, ib)
                aug_host[0, us * 128 : (us + 1) * 128] = 1.0
                aug_host[1, us * 128 : (us + 1) * 128] = -0.5 * sqf[r0 : r0 + 128]
                if 2 <= t <= 8:
                    w_host[:, (t - 2) * NIB + ib, 0, :] = xt[0:128, r0 : r0 + 128]
                    w_host[:, (t - 2) * NIB + ib, 1, :] = xt[128:256, r0 : r0 + 128]
        in_maps.append({"xT": rhs_host, "wT": w_host, "aug2": aug_host})
    return in_maps


def _combine(results):
    beta, c, host_terms = _CACHE["fit"]
    total = 0.0
    for k in range(NCORES):
        r = np.asarray(results[k]["res"], np.float64)
        m = r[:, :NTILES].sum(axis=0)  # [NTILES]
        for t in (0, 1):  # add back the mirrored quadrants of the diag tiles
            m[t] += r[:, NTILES + 2 * t].sum() + r[:, NTILES + 2 * t + 1].sum()
        w = np.array([wt for (_rb, _cb, wt) in _core_tiles(k)])
        total += float(np.dot(w, c[2] * m))
    return np.float32(total / (B * B) + host_terms)


def kernel(source_features, target_features):
    from concourse.bass_utils import run_bass_kernel_spmd

    in_maps = _host_prep(source_features, target_features)
    nc = _get_program()
    out = run_bass_kernel_spmd(nc, in_maps, list(range(NCORES)))
    return _combine(out.results)
